# revision 3
# baseline (speedup 1.0000x reference)
# Trainium2 Bass kernel for nn_Attention_60464549593105.
#
# Math (per batch b, spatial point (h,w), seq s):
#   energy[k] = tanh( We @ enc[:,s] + Wh @ hidden + b_att )      (K=128)
#   score[s]  = W_v . energy
#   out[s]    = softmax_s(score)
#
# Strategy: shard the H axis across 8 cores (8 rows each) so softmax over
# seq is core-local (no collectives). Each core streams its 64 MiB slice of
# encoder_outputs once -> memory-bound; all compute engines are kept under
# the DMA roofline:
#   - proj_e: PE matmul lhsT=We^T [E,K], rhs=enc chunk [E, 512]
#   - +proj_h: alternate between a second accumulating PE matmul (even s)
#     and a DVE add (odd s) to balance PE vs DVE occupancy
#   - tanh(+b_att): one ACT pass, bias port carries b_att
#   - scores: matvec with a sliding-window masked W_v stationary operand so
#     the 64 per-s results accumulate directly into a [64, 512] psum tile
#     (partition = s) -> softmax-ready layout
#   - softmax over s: exp (ACT; max-subtraction skipped, |score| <=
#     sum|W_v| ~ 5 so exp is safe in fp32), all-ones [64,64] matmul =
#     sum-over-partitions broadcast to all 64 rows in one op, DVE
#     reciprocal + multiply.

import numpy as np

B, D, E, S, H, W = 4, 128, 128, 64, 64, 64
K = 128
NCORES = 8
HSH = H // NCORES          # h rows per core
FREE = HSH * W             # free-dim elements per (b, s) tile
SCH = 8                    # seq positions per DMA chunk (2 MiB per DMA)

_CACHE = {}


def _build_bass():
    import concourse.bacc as bacc
    import concourse.mybir as mybir
    import concourse.tile as tile
    from contextlib import ExitStack

    f32 = mybir.dt.float32
    AF = mybir.ActivationFunctionType

    nc = bacc.Bacc("TRN2", target_bir_lowering=False, debug=False)
    enc = nc.dram_tensor("enc", [B, E, S * FREE], f32, kind="ExternalInput")
    hid = nc.dram_tensor("hid", [B, D, FREE], f32, kind="ExternalInput")
    weT = nc.dram_tensor("weT", [E, K], f32, kind="ExternalInput")
    whT = nc.dram_tensor("whT", [D, K], f32, kind="ExternalInput")
    batt = nc.dram_tensor("batt", [K, 1], f32, kind="ExternalInput")
    wvs = nc.dram_tensor("wvs", [K, 2 * S], f32, kind="ExternalInput")
    out = nc.dram_tensor("out", [B, S, FREE], f32, kind="ExternalOutput")

    with tile.TileContext(nc) as tc, ExitStack() as ctx:
        consts = ctx.enter_context(tc.tile_pool(name="consts", bufs=1))
        encp = ctx.enter_context(tc.tile_pool(name="encp", bufs=3))
        epsum = ctx.enter_context(tc.tile_pool(name="epsum", bufs=3, space="PSUM"))
        spsum = ctx.enter_context(tc.tile_pool(name="spsum", bufs=2, space="PSUM"))
        bpsum = ctx.enter_context(tc.tile_pool(name="bpsum", bufs=1, space="PSUM"))
        work = ctx.enter_context(tc.tile_pool(name="work", bufs=4))
        smax = ctx.enter_context(tc.tile_pool(name="smax", bufs=2))

        weT_sb = consts.tile([E, K], f32)
        nc.sync.dma_start(out=weT_sb, in_=weT[:])
        whT_sb = consts.tile([D, K], f32)
        nc.sync.dma_start(out=whT_sb, in_=whT[:])
        batt_sb = consts.tile([K, 1], f32)
        nc.sync.dma_start(out=batt_sb, in_=batt[:])
        wvs_sb = consts.tile([K, 2 * S], f32)
        nc.sync.dma_start(out=wvs_sb, in_=wvs[:])
        ones_sb = consts.tile([S, S], f32)
        nc.vector.memset(ones_sb, 1.0)

        hid_sb = consts.tile([D, B * FREE], f32)
        for b in range(B):
            nc.sync.dma_start(
                out=hid_sb[:, b * FREE : (b + 1) * FREE], in_=hid[b]
            )

        # proj_h[b] = Wh @ hidden[b]  (once per b; reused by the DVE-add path)
        projh_sb = consts.tile([K, B * FREE], f32)
        for b in range(B):
            ph_ps = epsum.tile([K, FREE], f32, tag="e_ps", name="ph_ps")
            nc.tensor.matmul(
                ph_ps,
                lhsT=whT_sb,
                rhs=hid_sb[:, b * FREE : (b + 1) * FREE],
                start=True,
                stop=True,
            )
            nc.vector.tensor_copy(projh_sb[:, b * FREE : (b + 1) * FREE], ph_ps)

        for b in range(B):
            hslice = hid_sb[:, b * FREE : (b + 1) * FREE]
            pslice = projh_sb[:, b * FREE : (b + 1) * FREE]
            sc_ps = spsum.tile([S, FREE], f32, tag="sc_ps", name="sc_ps")
            for sc in range(S // SCH):
                et = encp.tile([E, SCH * FREE], f32, tag="et", name="et")
                off = sc * SCH * FREE
                nc.sync.dma_start(out=et, in_=enc[b, :, off : off + SCH * FREE])
                for j in range(SCH):
                    s = sc * SCH + j
                    rhs = et[:, j * FREE : (j + 1) * FREE]
                    e_ps = epsum.tile([K, FREE], f32, tag="e_ps", name="e_ps")
                    if s % 2 == 0:
                        nc.tensor.matmul(e_ps, lhsT=weT_sb, rhs=rhs, start=True, stop=False)
                        nc.tensor.matmul(e_ps, lhsT=whT_sb, rhs=hslice, start=False, stop=True)
                    else:
                        nc.tensor.matmul(e_ps, lhsT=weT_sb, rhs=rhs, start=True, stop=True)
                        nc.vector.tensor_add(out=e_ps, in0=e_ps, in1=pslice)
                    th = work.tile([K, FREE], f32, tag="th", name="th")
                    nc.scalar.activation(th, e_ps, AF.Tanh, bias=batt_sb)
                    # lhsT column j of the slice is W_v iff j == s, else 0:
                    # accumulates W_v . tanh into psum partition s.
                    nc.tensor.matmul(
                        sc_ps,
                        lhsT=wvs_sb[:, (S - 1) - s : (2 * S - 1) - s],
                        rhs=th,
                        start=(s == 0),
                        stop=(s == S - 1),
                    )
            expv = smax.tile([S, FREE], f32, tag="expv", name="expv")
            nc.scalar.activation(expv, sc_ps, AF.Exp)
            sum_bc = bpsum.tile([S, FREE], f32, tag="sum_bc", name="sum_bc")
            nc.tensor.matmul(sum_bc, lhsT=ones_sb, rhs=expv, start=True, stop=True)
            rec = smax.tile([S, FREE], f32, tag="rec", name="rec")
            nc.vector.reciprocal(rec, sum_bc)
            ob = smax.tile([S, FREE], f32, tag="ob", name="ob")
            nc.vector.tensor_mul(ob, expv, rec)
            nc.sync.dma_start(out=out[b], in_=ob)
    nc.compile()
    return nc


def _get_bass():
    if "nc" not in _CACHE:
        _CACHE["nc"] = _build_bass()
    return _CACHE["nc"]


def kernel(hidden_state, encoder_outputs, W_att, b_att, W_v):
    from concourse.bass_utils import run_bass_kernel_spmd

    hidden_state = np.asarray(hidden_state, dtype=np.float32)
    encoder_outputs = np.asarray(encoder_outputs, dtype=np.float32)
    W_att = np.asarray(W_att, dtype=np.float32)
    b_att = np.asarray(b_att, dtype=np.float32)
    W_v = np.asarray(W_v, dtype=np.float32)

    weT = np.ascontiguousarray(W_att[:, D:].T)      # [E, K]
    whT = np.ascontiguousarray(W_att[:, :D].T)      # [D, K]
    batt = np.ascontiguousarray(b_att.reshape(K, 1))
    wvs = np.zeros((K, 2 * S), dtype=np.float32)
    wvs[:, S - 1] = W_v[0]

    in_maps = []
    for c in range(NCORES):
        h0 = c * HSH
        enc_c = np.ascontiguousarray(
            encoder_outputs[:, :, :, h0 : h0 + HSH, :]
        ).reshape(B, E, S * FREE)
        hid_c = np.ascontiguousarray(
            hidden_state[:, :, h0 : h0 + HSH, :]
        ).reshape(B, D, FREE)
        in_maps.append(
            {"enc": enc_c, "hid": hid_c, "weT": weT, "whT": whT,
             "batt": batt, "wvs": wvs}
        )

    nc = _get_bass()
    kwargs = dict(_CACHE.get("run_kwargs", {}))
    res = run_bass_kernel_spmd(nc, in_maps, core_ids=list(range(NCORES)), **kwargs)
    _CACHE["last_result"] = res
    shards = [r["out"].reshape(B, S, HSH, W) for r in res.results]
    return np.concatenate(shards, axis=2)


# revision 4
# speedup vs baseline: 2.5030x; 2.5030x over previous
# Trainium2 Bass kernel for nn_Attention_60464549593105.
#
# Math (per batch b, spatial point (h,w), seq s):
#   energy[k] = tanh( We @ enc[:,s] + Wh @ hidden + b_att )      (K=128)
#   score[s]  = W_v . energy
#   out[s]    = softmax_s(score)
#
# Strategy: shard the H axis across 8 cores (8 rows each) so softmax over
# seq is core-local (no collectives). Each core streams its 64 MiB slice of
# encoder_outputs once -> memory-bound target; compute engines kept under
# the DMA roofline:
#   - all large matmuls run in float32r (bf16 hi/lo pair, ~1.5e-5/element
#     rounding; measured 1.6e-4 rel on a 128-deep contraction) which streams
#     at 1 cycle/row vs fp32's LOW_HIGH 4 cycles/row. The fp32->fp32r
#     rounding is free: SWDGE DMA casts during the HBM load, and ACT writes
#     its tanh output as fp32r directly.
#   - proj_e: PE matmul lhsT=We^T [E,K], rhs=enc chunk [E, 512]
#   - +proj_h: alternate between a second accumulating PE matmul and a DVE
#     add of a precomputed proj_h tile to balance PE vs DVE occupancy
#   - tanh(+b_att): one ACT pass (psum -> sbuf fp32r), bias port = b_att
#   - scores: matvec with a sliding-window masked W_v stationary operand so
#     the 64 per-s results accumulate directly into a [64, 512] psum tile
#     (partition = s) -> softmax-ready layout
#   - softmax over s in plain fp32: exp (ACT; max-subtraction skipped,
#     |score| <= sum|W_v| ~ 5 so exp is safe in fp32), all-ones [64,64]
#     matmul = sum-over-partitions broadcast to all 64 rows in one op, DVE
#     reciprocal + multiply.

import numpy as np

B, D, E, S, H, W = 4, 128, 128, 64, 64, 64
K = 128
NCORES = 8
HSH = H // NCORES          # h rows per core
FREE = HSH * W             # free-dim elements per (b, s) tile
SCH = 16                   # seq positions per enc DMA chunk (4 MiB per DMA)
PE_ADD_MOD = 2             # s % 4 < MOD -> proj_h via PE accumulate, else DVE

_CACHE = {}


def _build_bass():
    import concourse.bacc as bacc
    import concourse.mybir as mybir
    import concourse.tile as tile
    from contextlib import ExitStack

    f32 = mybir.dt.float32
    f32r = mybir.dt.float32r
    AF = mybir.ActivationFunctionType

    nc = bacc.Bacc("TRN2", target_bir_lowering=False, debug=False)
    enc = nc.dram_tensor("enc", [B, E, S * FREE], f32, kind="ExternalInput")
    hid = nc.dram_tensor("hid", [B, D, FREE], f32, kind="ExternalInput")
    weT = nc.dram_tensor("weT", [E, K], f32, kind="ExternalInput")
    whT = nc.dram_tensor("whT", [D, K], f32, kind="ExternalInput")
    batt = nc.dram_tensor("batt", [K, 1], f32, kind="ExternalInput")
    wvs = nc.dram_tensor("wvs", [K, 2 * S], f32, kind="ExternalInput")
    out = nc.dram_tensor("out", [B, S, FREE], f32, kind="ExternalOutput")

    with tile.TileContext(nc) as tc, ExitStack() as ctx:
        consts = ctx.enter_context(tc.tile_pool(name="consts", bufs=1))
        encp = ctx.enter_context(tc.tile_pool(name="encp", bufs=3))
        epsum = ctx.enter_context(tc.tile_pool(name="epsum", bufs=3, space="PSUM"))
        spsum = ctx.enter_context(tc.tile_pool(name="spsum", bufs=2, space="PSUM"))
        bpsum = ctx.enter_context(tc.tile_pool(name="bpsum", bufs=1, space="PSUM"))
        work = ctx.enter_context(tc.tile_pool(name="work", bufs=4))
        smax = ctx.enter_context(tc.tile_pool(name="smax", bufs=2))

        # fp32r constants arrive pre-rounded via SWDGE cast-DMA
        weT_sb = consts.tile([E, K], f32r)
        nc.gpsimd.dma_start(out=weT_sb, in_=weT[:])
        whT_sb = consts.tile([D, K], f32r)
        nc.gpsimd.dma_start(out=whT_sb, in_=whT[:])
        wvs_sb = consts.tile([K, 2 * S], f32r)
        nc.gpsimd.dma_start(out=wvs_sb, in_=wvs[:])
        batt_sb = consts.tile([K, 1], f32)
        nc.sync.dma_start(out=batt_sb, in_=batt[:])
        ones_f32 = consts.tile([S, S], f32)
        nc.vector.memset(ones_f32, 1.0)

        hid_sb = consts.tile([D, B * FREE], f32r)
        for b in range(B):
            nc.gpsimd.dma_start(
                out=hid_sb[:, b * FREE : (b + 1) * FREE], in_=hid[b]
            )

        # proj_h[b] = Wh @ hidden[b]  (once per b; reused by the DVE-add path)
        projh_sb = consts.tile([K, B * FREE], f32)
        for b in range(B):
            ph_ps = epsum.tile([K, FREE], f32, tag="e_ps", name="ph_ps")
            nc.tensor.matmul(
                ph_ps,
                lhsT=whT_sb,
                rhs=hid_sb[:, b * FREE : (b + 1) * FREE],
                start=True,
                stop=True,
            )
            nc.vector.tensor_copy(projh_sb[:, b * FREE : (b + 1) * FREE], ph_ps)

        for b in range(B):
            hslice = hid_sb[:, b * FREE : (b + 1) * FREE]
            pslice = projh_sb[:, b * FREE : (b + 1) * FREE]
            sc_ps = spsum.tile([S, FREE], f32, tag="sc_ps", name="sc_ps")
            for sc in range(S // SCH):
                et = encp.tile([E, SCH * FREE], f32r, tag="et", name="et")
                off = sc * SCH * FREE
                nc.gpsimd.dma_start(out=et, in_=enc[b, :, off : off + SCH * FREE])
                for j in range(SCH):
                    s = sc * SCH + j
                    rhs = et[:, j * FREE : (j + 1) * FREE]
                    e_ps = epsum.tile([K, FREE], f32, tag="e_ps", name="e_ps")
                    if s % 4 < PE_ADD_MOD:
                        nc.tensor.matmul(e_ps, lhsT=weT_sb, rhs=rhs, start=True, stop=False)
                        nc.tensor.matmul(e_ps, lhsT=whT_sb, rhs=hslice, start=False, stop=True)
                    else:
                        nc.tensor.matmul(e_ps, lhsT=weT_sb, rhs=rhs, start=True, stop=True)
                        nc.vector.tensor_add(out=e_ps, in0=e_ps, in1=pslice)
                    th = work.tile([K, FREE], f32r, tag="th", name="th")
                    nc.scalar.activation(th, e_ps, AF.Tanh, bias=batt_sb)
                    # lhsT column j of the slice is W_v iff j == s, else 0:
                    # accumulates W_v . tanh into psum partition s.
                    nc.tensor.matmul(
                        sc_ps,
                        lhsT=wvs_sb[:, (S - 1) - s : (2 * S - 1) - s],
                        rhs=th,
                        start=(s == 0),
                        stop=(s == S - 1),
                    )
            expv = smax.tile([S, FREE], f32, tag="expv", name="expv")
            nc.scalar.activation(expv, sc_ps, AF.Exp)
            sum_bc = bpsum.tile([S, FREE], f32, tag="sum_bc", name="sum_bc")
            nc.tensor.matmul(sum_bc, lhsT=ones_f32, rhs=expv, start=True, stop=True)
            rec = smax.tile([S, FREE], f32, tag="rec", name="rec")
            nc.vector.reciprocal(rec, sum_bc)
            ob = smax.tile([S, FREE], f32, tag="ob", name="ob")
            nc.vector.tensor_mul(ob, expv, rec)
            nc.sync.dma_start(out=out[b], in_=ob)
    nc.compile()
    return nc


def _get_bass():
    if "nc" not in _CACHE:
        _CACHE["nc"] = _build_bass()
    return _CACHE["nc"]


def kernel(hidden_state, encoder_outputs, W_att, b_att, W_v):
    from concourse.bass_utils import run_bass_kernel_spmd

    hidden_state = np.asarray(hidden_state, dtype=np.float32)
    encoder_outputs = np.asarray(encoder_outputs, dtype=np.float32)
    W_att = np.asarray(W_att, dtype=np.float32)
    b_att = np.asarray(b_att, dtype=np.float32)
    W_v = np.asarray(W_v, dtype=np.float32)

    weT = np.ascontiguousarray(W_att[:, D:].T)      # [E, K]
    whT = np.ascontiguousarray(W_att[:, :D].T)      # [D, K]
    batt = np.ascontiguousarray(b_att.reshape(K, 1))
    wvs = np.zeros((K, 2 * S), dtype=np.float32)
    wvs[:, S - 1] = W_v[0]

    in_maps = []
    for c in range(NCORES):
        h0 = c * HSH
        enc_c = np.ascontiguousarray(
            encoder_outputs[:, :, :, h0 : h0 + HSH, :]
        ).reshape(B, E, S * FREE)
        hid_c = np.ascontiguousarray(
            hidden_state[:, :, h0 : h0 + HSH, :]
        ).reshape(B, D, FREE)
        in_maps.append(
            {"enc": enc_c, "hid": hid_c, "weT": weT, "whT": whT,
             "batt": batt, "wvs": wvs}
        )

    nc = _get_bass()
    kwargs = dict(_CACHE.get("run_kwargs", {}))
    res = run_bass_kernel_spmd(nc, in_maps, core_ids=list(range(NCORES)), **kwargs)
    _CACHE["last_result"] = res
    shards = [r["out"].reshape(B, S, HSH, W) for r in res.results]
    return np.concatenate(shards, axis=2)
